# revision 4
# baseline (speedup 1.0000x reference)
"""Embedding lookup kernel for Trainium2 (8 NeuronCores, SPMD).

Strategy: token-parallel gather. The reference one-hot+matmul is just an
embedding row gather: out[b, s, :] = weight[x[b, s], :].

- Flatten x [2, 4096] -> [8192] tokens; each of the 8 cores handles 1024
  contiguous tokens.
- Each core receives the FULL weight table [32000, 128] f32 in its DRAM and
  its 1024 indices as int16 (vocab 32000 < 2^15) in dma_gather's wrapped
  layout: token k's index at [k % 16, k // 16], replicated across the 8
  groups of 16 partitions (one per GpSimd Q7 core).
- One InstDMAGatherAnt gathers all 1024 rows (512 B each) into an SBUF tile
  [128, 8, 128] f32: token k lands at partition k % 128, slot k // 128.
- One direct DMA writes the tile contiguously to DRAM out [128, 1024]
  (4 KiB per partition); the host de-interleaves (transpose is free there).

No collectives; the host concatenates the 8 per-core outputs.
"""

import numpy as np

import concourse.tile as tile
from concourse import bacc, mybir
from concourse.bass_utils import run_bass_kernel_spmd

N_CORES = 8
B, S = 2, 4096
VOCAB, DIM = 32000, 128
P = 128
TOKENS = B * S                      # 8192
TPC = TOKENS // N_CORES             # 1024 tokens per core
SLOTS = TPC // P                    # 8 row-slots per partition


def build_nc():
    nc = bacc.Bacc(None, target_bir_lowering=False)
    x = nc.dram_tensor("x", [P, TPC // 16], mybir.dt.int16, kind="ExternalInput")
    w = nc.dram_tensor("weight", [VOCAB, DIM], mybir.dt.float32, kind="ExternalInput")
    out = nc.dram_tensor("out", [P, TPC * DIM // P], mybir.dt.float32, kind="ExternalOutput")

    with tile.TileContext(nc) as tc:
        with tc.tile_pool(name="sbuf", bufs=1) as pool:
            idx_tile = pool.tile([P, TPC // 16], mybir.dt.int16)
            nc.sync.dma_start(out=idx_tile[:], in_=x[:])
            g = pool.tile([P, SLOTS, DIM], mybir.dt.float32)
            nc.gpsimd.dma_gather(
                out_ap=g[:],
                in_ap=w[:],
                idxs_ap=idx_tile[:],
                num_idxs=TPC,
                num_idxs_reg=TPC,
                elem_size=DIM,
            )
            nc.sync.dma_start(
                out=out[:].rearrange("p (j d) -> p j d", j=SLOTS), in_=g[:]
            )
    nc.compile()
    return nc


def _wrap_idx(tok_idx: np.ndarray) -> np.ndarray:
    """[1024] int -> [128, 64] int16, dma_gather wrapped + replicated layout."""
    t = tok_idx.astype(np.int16).reshape(TPC // 16, 16).T
    return np.ascontiguousarray(np.tile(t, (8, 1)))


_NC_CACHE = None


def kernel(x: np.ndarray, weight: np.ndarray, **run_kwargs):
    global _NC_CACHE
    if _NC_CACHE is None:
        _NC_CACHE = build_nc()
    nc = _NC_CACHE

    x_flat = np.asarray(x).reshape(-1).astype(np.int32)
    w = np.ascontiguousarray(np.asarray(weight, dtype=np.float32))

    in_maps = [
        {
            "x": _wrap_idx(x_flat[c * TPC : (c + 1) * TPC]),
            "weight": w,
        }
        for c in range(N_CORES)
    ]
    res = run_bass_kernel_spmd(nc, in_maps, core_ids=list(range(N_CORES)), **run_kwargs)
    parts = [
        res.results[c]["out"].reshape(P, SLOTS, DIM).transpose(1, 0, 2).reshape(TPC, DIM)
        for c in range(N_CORES)
    ]
    full = np.concatenate(parts, axis=0).reshape(B, S, DIM)
    if run_kwargs:
        return full, res
    return full


# revision 5
# speedup vs baseline: 1.3535x; 1.3535x over previous
"""Embedding lookup kernel for Trainium2 (8 NeuronCores, SPMD).

Strategy: token-parallel gather (an embedding lookup IS a row gather:
out[b, s, :] = weight[x[b, s], :]).

- Flatten x [2, 4096] -> [8192] tokens; each of the 8 cores handles 1024
  contiguous tokens. Each core gets the FULL weight table in its DRAM.
- Per core (raw Bacc program, no Tile framework overhead):
    1. HWDGE DMA loads the 1024 indices as [128, 8] int32 into SBUF
       (partition p holds tokens p*8 .. p*8+7).
    2. 8 SWDGE indirect DMAs (one per token column j) gather 128 rows each
       (one index per partition) into an SBUF tile column [128, 128] f32.
    3. As each gather's completion semaphore fires, an HWDGE DMA writes that
       column back to DRAM out[:, j*128:(j+1)*128] — writeback overlaps the
       remaining gathers' descriptor generation.
- out [128, 1024] f32 reshapes host-side to [1024, 128] (token p*8+j at
  partition p, col-block j). Host concatenates the 8 per-core outputs.

No collectives. SWDGE descriptor generation (~1.4us per 128-row gather) is
the dominant cost; data movement (512 KiB in + 512 KiB out per core) hides
under it.
"""

import contextlib

import numpy as np

import concourse.bass as bass
from concourse import bacc, mybir
from concourse.bass_utils import run_bass_kernel_spmd

N_CORES = 8
B, S = 2, 4096
VOCAB, DIM = 32000, 128
P = 128
TOKENS = B * S                      # 8192
TPC = TOKENS // N_CORES             # 1024 tokens per core
TPP = TPC // P                      # 8 tokens per partition


def build_nc():
    nc = bacc.Bacc(None, target_bir_lowering=False)
    x = nc.dram_tensor("x", [P, TPP], mybir.dt.int32, kind="ExternalInput")
    w = nc.dram_tensor("weight", [VOCAB, DIM], mybir.dt.float32, kind="ExternalInput")
    out = nc.dram_tensor("out", [P, TPC], mybir.dt.float32, kind="ExternalOutput")

    with contextlib.ExitStack() as ctx:
        idx_tile = ctx.enter_context(
            nc.sbuf_tensor("idx_tile", [P, TPP], mybir.dt.int32)
        )
        g = ctx.enter_context(nc.sbuf_tensor("g", [P, TPC], mybir.dt.float32))
        s_idx = ctx.enter_context(nc.semaphore("s_idx"))
        s_out = ctx.enter_context(nc.semaphore("s_out"))
        s_gs = [ctx.enter_context(nc.semaphore(f"s_g{j}")) for j in range(TPP)]

        nc.sync.dma_start(idx_tile[:], x[:]).then_inc(s_idx, 16)
        nc.gpsimd.wait_ge(s_idx, 16)
        for j in range(TPP):
            nc.gpsimd.indirect_dma_start(
                out=g[:, j * DIM : (j + 1) * DIM],
                out_offset=None,
                in_=w[:],
                in_offset=bass.IndirectOffsetOnAxis(ap=idx_tile[:, j : j + 1], axis=0),
            ).then_inc(s_gs[j], 16)
        for j in range(TPP):
            nc.sync.wait_ge(s_gs[j], 16)
            nc.sync.dma_start(
                out[:, j * DIM : (j + 1) * DIM], g[:, j * DIM : (j + 1) * DIM]
            ).then_inc(s_out, 16)
        nc.sync.wait_ge(s_out, 16 * TPP)
    nc.compile()
    return nc


_NC_CACHE = None


def kernel(x: np.ndarray, weight: np.ndarray, **run_kwargs):
    global _NC_CACHE
    if _NC_CACHE is None:
        _NC_CACHE = build_nc()
    nc = _NC_CACHE

    x_flat = np.asarray(x).reshape(-1).astype(np.int32)
    w = np.ascontiguousarray(np.asarray(weight, dtype=np.float32))

    in_maps = [
        {
            "x": np.ascontiguousarray(x_flat[c * TPC : (c + 1) * TPC].reshape(P, TPP)),
            "weight": w,
        }
        for c in range(N_CORES)
    ]
    res = run_bass_kernel_spmd(nc, in_maps, core_ids=list(range(N_CORES)), **run_kwargs)
    # out [128, 1024] -> [1024, 128]: token p*TPP+j lives at [p, j*DIM:(j+1)*DIM]
    parts = [res.results[c]["out"].reshape(TPC, DIM) for c in range(N_CORES)]
    full = np.concatenate(parts, axis=0).reshape(B, S, DIM)
    if run_kwargs:
        return full, res
    return full


# revision 7
# speedup vs baseline: 1.3809x; 1.0203x over previous
"""Embedding lookup kernel for Trainium2 (8 NeuronCores, SPMD).

Strategy: token-parallel gather (an embedding lookup IS a row gather:
out[b, s, :] = weight[x[b, s], :]).

- Flatten x [2, 4096] -> [8192] tokens; each of the 8 cores handles 1024
  contiguous tokens. Each core gets the FULL weight table in its DRAM.
- Per core (raw Bacc program, no Tile framework overhead):
    1. HWDGE DMA loads the 1024 indices as [128, 8] int32 into SBUF
       (partition p holds tokens p*8 .. p*8+7), split in two (cols 0-1,
       cols 2-7) so the first gathers start as soon as their columns land.
    2. 8 SWDGE indirect DMAs (one per token column j) gather 128 rows each
       (one index per partition) into an SBUF tile column [128, 128] f32.
    3. As each gather's completion semaphore fires, an HWDGE DMA writes that
       column back to DRAM out[:, j*128:(j+1)*128] — writeback overlaps the
       remaining gathers' descriptor generation. No final completion wait:
       the NEFF epilogue's engine drains already block until the HWDGE
       queues are empty (verified bit-exact on HW).
- out [128, 1024] f32 reshapes host-side to [1024, 128] (token p*8+j at
  partition p, col-block j). Host concatenates the 8 per-core outputs.

No collectives. SWDGE descriptor generation (~1.4us per 128-row gather) is
the dominant cost; data movement (512 KiB in + 512 KiB out per core) hides
under it.
"""

import contextlib

import numpy as np

import concourse.bass as bass
from concourse import bacc, mybir
from concourse.bass_utils import run_bass_kernel_spmd

N_CORES = 8
B, S = 2, 4096
VOCAB, DIM = 32000, 128
P = 128
TOKENS = B * S                      # 8192
TPC = TOKENS // N_CORES             # 1024 tokens per core
TPP = TPC // P                      # 8 tokens per partition


def build_nc():
    nc = bacc.Bacc(None, target_bir_lowering=False)
    x = nc.dram_tensor("x", [P, TPP], mybir.dt.int32, kind="ExternalInput")
    w = nc.dram_tensor("weight", [VOCAB, DIM], mybir.dt.float32, kind="ExternalInput")
    out = nc.dram_tensor("out", [P, TPC], mybir.dt.float32, kind="ExternalOutput")

    with contextlib.ExitStack() as ctx:
        idx_tile = ctx.enter_context(
            nc.sbuf_tensor("idx_tile", [P, TPP], mybir.dt.int32)
        )
        g = ctx.enter_context(nc.sbuf_tensor("g", [P, TPC], mybir.dt.float32))
        s_idx = ctx.enter_context(nc.semaphore("s_idx"))
        s_idx2 = ctx.enter_context(nc.semaphore("s_idx2"))
        s_out = ctx.enter_context(nc.semaphore("s_out"))
        s_gs = [ctx.enter_context(nc.semaphore(f"s_g{j}")) for j in range(TPP)]

        nc.sync.dma_start(idx_tile[:, :2], x[:, :2]).then_inc(s_idx, 16)
        nc.sync.dma_start(idx_tile[:, 2:], x[:, 2:]).then_inc(s_idx2, 16)
        nc.gpsimd.wait_ge(s_idx, 16)
        for j in range(TPP):
            if j == 2:
                nc.gpsimd.wait_ge(s_idx2, 16)
            nc.gpsimd.indirect_dma_start(
                out=g[:, j * DIM : (j + 1) * DIM],
                out_offset=None,
                in_=w[:],
                in_offset=bass.IndirectOffsetOnAxis(ap=idx_tile[:, j : j + 1], axis=0),
            ).then_inc(s_gs[j], 16)
        for j in range(TPP):
            nc.sync.wait_ge(s_gs[j], 16)
            nc.sync.dma_start(
                out[:, j * DIM : (j + 1) * DIM], g[:, j * DIM : (j + 1) * DIM]
            ).then_inc(s_out, 16)
    nc.compile()
    return nc


_NC_CACHE = None


def kernel(x: np.ndarray, weight: np.ndarray, **run_kwargs):
    global _NC_CACHE
    if _NC_CACHE is None:
        _NC_CACHE = build_nc()
    nc = _NC_CACHE

    x_flat = np.asarray(x).reshape(-1).astype(np.int32)
    w = np.ascontiguousarray(np.asarray(weight, dtype=np.float32))

    in_maps = [
        {
            "x": np.ascontiguousarray(x_flat[c * TPC : (c + 1) * TPC].reshape(P, TPP)),
            "weight": w,
        }
        for c in range(N_CORES)
    ]
    res = run_bass_kernel_spmd(nc, in_maps, core_ids=list(range(N_CORES)), **run_kwargs)
    # out [128, 1024] -> [1024, 128]: token p*TPP+j lives at [p, j*DIM:(j+1)*DIM]
    parts = [res.results[c]["out"].reshape(TPC, DIM) for c in range(N_CORES)]
    full = np.concatenate(parts, axis=0).reshape(B, S, DIM)
    if run_kwargs:
        return full, res
    return full


# revision 8
# speedup vs baseline: 1.4855x; 1.0757x over previous
"""Embedding lookup kernel for Trainium2 (8 NeuronCores, SPMD).

Strategy: token-parallel gather (an embedding lookup IS a row gather:
out[b, s, :] = weight[x[b, s], :]).

- Flatten x [2, 4096] -> [8192] tokens; each of the 8 cores handles 1024
  contiguous tokens. Each core gets the FULL weight table in its DRAM.
- Per core (raw Bacc program, no Tile framework overhead; the Bass entry
  all-engine barrier is skipped — every cross-engine dependency below is
  ordered by an explicit semaphore, so each engine only needs its own
  program order):
    1. HWDGE DMA loads the 1024 indices as [128, 8] int32 into SBUF
       (partition p holds tokens p*8 .. p*8+7), split in two (cols 0-1,
       cols 2-7) so the first gathers start as soon as their columns land.
    2. While that DMA's ~2us HBM-read + completion latency elapses, a dummy
       warmup indirect DMA (indices from a memset-zero tile) runs on the
       Pool engine so the first real gather executes at steady-state cost.
    3. 8 SWDGE indirect DMAs (one per token column j) gather 128 rows each
       (one index per partition — a hard HW limit) into an SBUF tile column
       [128, 128] f32.  SWDGE descriptor generation (~1.1us/op on Q7 pair
       0, serial) is the dominant cost; the 512 KiB of gather traffic
       drains under it.
    4. As each gather's completion semaphore fires, an HWDGE DMA writes
       that column back to DRAM out[:, j*128:(j+1)*128], overlapping the
       remaining gathers. No final completion wait: the NEFF epilogue's
       engine drains already block until the HWDGE queues are empty
       (verified bit-exact on HW).
- out [128, 1024] f32 reshapes host-side to [1024, 128] (token p*8+j at
  partition p, col-block j). Host concatenates the 8 per-core outputs.

No collectives. Measured ~23.4us exec (neuron-profile), bit-exact vs the
one-hot matmul reference.
"""

import contextlib

import numpy as np

import concourse.bass as bass
from concourse import bacc, mybir
from concourse.bass_utils import run_bass_kernel_spmd

N_CORES = 8
B, S = 2, 4096
VOCAB, DIM = 32000, 128
P = 128
TOKENS = B * S                      # 8192
TPC = TOKENS // N_CORES             # 1024 tokens per core
TPP = TPC // P                      # 8 tokens per partition


def build_nc():
    # Skip the Bass-constructor entry barrier (gates the first DMA behind
    # all engines' init); restore the method right after construction.
    orig_barrier = bass.Bass.all_engine_barrier
    bass.Bass.all_engine_barrier = lambda self, *a, **k: None
    try:
        nc = bacc.Bacc(None, target_bir_lowering=False)
    finally:
        bass.Bass.all_engine_barrier = orig_barrier

    x = nc.dram_tensor("x", [P, TPP], mybir.dt.int32, kind="ExternalInput")
    w = nc.dram_tensor("weight", [VOCAB, DIM], mybir.dt.float32, kind="ExternalInput")
    out = nc.dram_tensor("out", [P, TPC], mybir.dt.float32, kind="ExternalOutput")

    with contextlib.ExitStack() as ctx:
        idx_tile = ctx.enter_context(
            nc.sbuf_tensor("idx_tile", [P, TPP], mybir.dt.int32)
        )
        g = ctx.enter_context(nc.sbuf_tensor("g", [P, TPC], mybir.dt.float32))
        dummy_idx = ctx.enter_context(
            nc.sbuf_tensor("dummy_idx", [P, 1], mybir.dt.int32)
        )
        scratch = ctx.enter_context(
            nc.sbuf_tensor("scratch", [P, DIM], mybir.dt.float32)
        )
        s_idx = ctx.enter_context(nc.semaphore("s_idx"))
        s_idx2 = ctx.enter_context(nc.semaphore("s_idx2"))
        s_out = ctx.enter_context(nc.semaphore("s_out"))
        s_warm = ctx.enter_context(nc.semaphore("s_warm"))
        s_ms = ctx.enter_context(nc.semaphore("s_ms"))
        s_gs = [ctx.enter_context(nc.semaphore(f"s_g{j}")) for j in range(TPP)]

        nc.sync.dma_start(idx_tile[:, :2], x[:, :2]).then_inc(s_idx, 16)
        nc.sync.dma_start(idx_tile[:, 2:], x[:, 2:]).then_inc(s_idx2, 16)

        # Warmup gather, hidden inside the idx-DMA latency window.
        nc.gpsimd.memset(dummy_idx[:], 0).then_inc(s_ms, 1)
        nc.gpsimd.wait_ge(s_ms, 1)
        nc.gpsimd.indirect_dma_start(
            out=scratch[:],
            out_offset=None,
            in_=w[:],
            in_offset=bass.IndirectOffsetOnAxis(ap=dummy_idx[:], axis=0),
        ).then_inc(s_warm, 16)

        nc.gpsimd.wait_ge(s_idx, 16)
        for j in range(TPP):
            if j == 2:
                nc.gpsimd.wait_ge(s_idx2, 16)
            nc.gpsimd.indirect_dma_start(
                out=g[:, j * DIM : (j + 1) * DIM],
                out_offset=None,
                in_=w[:],
                in_offset=bass.IndirectOffsetOnAxis(ap=idx_tile[:, j : j + 1], axis=0),
            ).then_inc(s_gs[j], 16)
        for j in range(TPP):
            nc.sync.wait_ge(s_gs[j], 16)
            nc.sync.dma_start(
                out[:, j * DIM : (j + 1) * DIM], g[:, j * DIM : (j + 1) * DIM]
            ).then_inc(s_out, 16)
    nc.compile()
    return nc


_NC_CACHE = None


def kernel(x: np.ndarray, weight: np.ndarray, **run_kwargs):
    global _NC_CACHE
    if _NC_CACHE is None:
        _NC_CACHE = build_nc()
    nc = _NC_CACHE

    x_flat = np.asarray(x).reshape(-1).astype(np.int32)
    w = np.ascontiguousarray(np.asarray(weight, dtype=np.float32))

    in_maps = [
        {
            "x": np.ascontiguousarray(x_flat[c * TPC : (c + 1) * TPC].reshape(P, TPP)),
            "weight": w,
        }
        for c in range(N_CORES)
    ]
    res = run_bass_kernel_spmd(nc, in_maps, core_ids=list(range(N_CORES)), **run_kwargs)
    # out [128, 1024] -> [1024, 128]: token p*TPP+j lives at [p, j*DIM:(j+1)*DIM]
    parts = [res.results[c]["out"].reshape(TPC, DIM) for c in range(N_CORES)]
    full = np.concatenate(parts, axis=0).reshape(B, S, DIM)
    if run_kwargs:
        return full, res
    return full
